# revision 34
# baseline (speedup 1.0000x reference)
"""CoAttention kernel for Trainium2 (8 NeuronCores, batch-parallel).

Math (per batch b):
    tm = t * mask_t[:, None]; fm = f * mask_f[:, None]
    S  = (tm @ W) @ fm.T                      # [LT, LF] bilinear scores
    alpha_t = softmax(tanh(rowmax(S)) + (mask_t-1)*BIG)
    alpha_f = softmax(tanh(colmax(S)) + (mask_f-1)*BIG)
    out = alpha_t @ tm + alpha_f @ fm

Bounded-error transformations (verified ~2.4e-3 relative vs the fp32
reference, tolerance 2e-2):
  - tanh is monotonic -> maxes taken on raw S, tanh applied to the
    max vectors only.
  - masking folds into the softmax bias: masked rows get bias -80 ->
    weight ~5e-35, which also covers the final weighted sums.
  - softmax max-subtraction dropped (tanh bounds values in [-1, 1]).
  - the score chain runs fp8-e4m3 with DoubleRow (2 MACs/cell/cycle):
    S only matters through "does tanh saturate", so fp8 noise is
    invisible; measured error identical to a bf16 score chain.
  - weighted sums stay bf16 (that's where all the error comes from).
  - masked rows are compacted out on the host (they are exact zeros in
    the reference's C and carry ~5e-35 softmax weight): unmasked rows
    pack to the front, padded with zero rows + bias -80 to lc =
    64*ceil(max_count/64) (320 for ~50% masks). Zero padding rows
    reproduce exactly the 0 entries the reference's masked rows
    contribute to the row/col maxes; a partial last 64-block is handled
    with partial-partition tiles (rm memset + m1 tail = -1e30 keep the
    unused lanes inert). The kernel variant for each lc is compiled
    lazily, so any mask density (up to full 512) stays correct.

Host-side prep is layout/dtype marshalling + the compaction gather;
all O(B*L*L*D) compute stays on device.

Per-core schedule (8 batches, software-pipelined), per iteration:
  PE: mm1(b) 8 DR MMs -> mm2(b) 6 DR MMs (g-interleaved across banks
  so projT evac latency hides) | colmax transposes(b-1) | weighted
  sums(b-2, bf16) | sum-of-exp matmul(b-1, ones[128,128] stationary so
  every partition holds the sums -> reciprocal is born replicated).
  DVE: per-tile rowmax + colmax max chain straight from PSUM.
  ACT: projT evac fp32->fp8, colmax chain seed, tanh, exp, 1/sum
  prescale, final out copy.
  All DMas are wide linear copies on one queue in priority order.
"""

import numpy as np
import ml_dtypes

import concourse.bass as bass
import concourse.tile as tile
from concourse import bacc, mybir
from concourse import masks as cmasks
from concourse.bass_utils import run_bass_kernel_spmd

F32 = mybir.dt.float32
BF16 = mybir.dt.bfloat16
F8 = mybir.dt.float8e4
AX = mybir.AxisListType
AF = mybir.ActivationFunctionType
DR = mybir.MatmulPerfMode.DoubleRow

N_CORES = 8
B, LT, LF, D = 64, 512, 512, 512
BL = B // N_CORES          # batches per core
P = 128                    # partitions
NB = D // P                # d-dim 128-blocks
LC_GRAN = 64               # compaction granularity (last block may be 64)
BIG = 80.0                 # mask bias (exp(-79) ~ 5e-35; ref uses 1e6, same result)


def _build(lc):
    nbl = (lc + P - 1) // P
    nc = bacc.Bacc("TRN2", target_bir_lowering=False, debug=False, num_devices=N_CORES)

    tT8_d = nc.dram_tensor("tT8", [P, BL, NB, lc], F8, kind="ExternalInput")
    fT8_d = nc.dram_tensor("fT8", [P, BL, NB, lc], F8, kind="ExternalInput")
    tbf_d = nc.dram_tensor("tbf", [P, BL, nbl, D], BF16, kind="ExternalInput")
    fbf_d = nc.dram_tensor("fbf", [P, BL, nbl, D], BF16, kind="ExternalInput")
    w8_d = nc.dram_tensor("w8", [P, NB, D], F8, kind="ExternalInput")
    bias_d = nc.dram_tensor("bias_tf", [P, BL, 2 * nbl], F32, kind="ExternalInput")
    o_d = nc.dram_tensor("out", [BL, D], F32, kind="ExternalOutput")

    with tile.TileContext(nc) as tc:
        _emit(tc, lc, tT8_d, fT8_d, tbf_d, fbf_d, w8_d, bias_d, o_d)
    nc.compile()
    return nc


def _emit(tc, lc, tT8_d, fT8_d, tbf_d, fbf_d, w8_d, bias_d, o_d):
    nc = tc.nc
    nbl = (lc + P - 1) // P
    with (
        tc.tile_pool(name="const", bufs=1) as cpool,
        tc.tile_pool(name="tf8", bufs=BL) as tf8_pool,
        tc.tile_pool(name="nat", bufs=BL) as nat_pool,
        tc.tile_pool(name="pjsb", bufs=2) as pj_sb_pool,
        tc.tile_pool(name="m1", bufs=2) as m1_pool,
        tc.tile_pool(name="sv", bufs=6) as sv_pool,
        tc.tile_pool(name="pjps", bufs=2, space="PSUM") as pj_ps_pool,
        tc.tile_pool(name="sps", bufs=4, space="PSUM") as s_ps_pool,
        tc.tile_pool(name="mtps", bufs=1, space="PSUM") as m1t_ps_pool,
        tc.tile_pool(name="fin", bufs=1, space="PSUM") as fin_ps_pool,
    ):
        pools = dict(
            tf8=tf8_pool, nat=nat_pool, pjsb=pj_sb_pool, m1=m1_pool,
            sv=sv_pool, pj_ps=pj_ps_pool, s_ps=s_ps_pool,
            m1t_ps=m1t_ps_pool, sm_ps=m1t_ps_pool, fin_ps=fin_ps_pool,
        )
        st = [dict(lc=lc, nbl=nbl) for _ in range(BL)]

        ident = cpool.tile([P, P], BF16)
        cmasks.make_identity(nc, ident[:])
        ones_sq = cpool.tile([P, P], BF16)
        nc.vector.memset(ones_sq[:], 1.0)

        # two DMA issue queues in explicit priority order: sync carries
        # the t-side, gpsimd the f-side, so batch 0's pair lands in half
        # the time; nat slabs (needed 2 stages later) trail by 2 batches.
        w8 = cpool.tile([P, NB, D], F8)
        nc.sync.dma_start(w8[:], w8_d.ap())
        bias_tf = cpool.tile([P, BL, 2 * nbl], F32)
        for b in range(BL):
            t8 = tf8_pool.tile([P, NB, lc], F8, tag="t8", name=f"t8_{b}")
            f8 = tf8_pool.tile([P, NB, lc], F8, tag="f8", name=f"f8_{b}")
            st[b]["t8"], st[b]["f8"] = t8, f8
            nat = nat_pool.tile([P, 2, nbl, D], BF16, tag="nat", name=f"nat_{b}")
            st[b]["nat"] = nat
        for b in range(BL):
            nc.sync.dma_start(st[b]["t8"][:], tT8_d.ap()[:, b])
            nc.sync.dma_start(st[b]["f8"][:], fT8_d.ap()[:, b])
            if b == 1:
                nc.sync.dma_start(bias_tf[:], bias_d.ap())
            if b >= 2:
                nat = st[b - 2]["nat"]
                nc.sync.dma_start(nat[:, 0], tbf_d.ap()[:, b - 2])
                nc.sync.dma_start(nat[:, 1], fbf_d.ap()[:, b - 2])
        for b in range(BL - 2, BL):
            nat = st[b]["nat"]
            nc.sync.dma_start(nat[:, 0], tbf_d.ap()[:, b])
            nc.sync.dma_start(nat[:, 1], fbf_d.ap()[:, b])

        consts = dict(w8=w8, ident=ident, ones_sq=ones_sq, bias_tf=bias_tf)
        out_acc = cpool.tile([1, BL, D], F32)
        consts["out_acc"] = out_acc

        for b in range(BL):
            ws_q = _ws_emitters(tc, b - 2, st[b - 2]) if b >= 2 else []
            _stage_mm1(tc, b, st[b], consts, pools, ws_q)
            _stage_mm2(tc, b, st[b], consts, pools, ws_q)
            if b >= 1:
                _stage_tr(tc, b - 1, st[b - 1], consts, pools)
            if b >= 2:
                _stage_out(tc, b - 2, st[b - 2], consts, pools, o_d)
            if b >= 1:
                _stage_sums(tc, b - 1, st[b - 1], consts, pools)
        _stage_tr(tc, BL - 1, st[BL - 1], consts, pools)
        for emit in _ws_emitters(tc, BL - 2, st[BL - 2]):
            emit()
        _stage_out(tc, BL - 2, st[BL - 2], consts, pools, o_d)
        _stage_sums(tc, BL - 1, st[BL - 1], consts, pools)
        for emit in _ws_emitters(tc, BL - 1, st[BL - 1]):
            emit()
        _stage_out(tc, BL - 1, st[BL - 1], consts, pools, o_d)


def _stage_mm1(tc, b, st, consts, pools, ws_q):
    """mm1: projT[e, l] = W.T @ tT (fp8 DoubleRow, contraction d)."""
    nc = tc.nc
    lc, nbl = st["lc"], st["nbl"]
    w8 = consts["w8"]
    t8 = st["t8"]

    projT = pools["pjsb"].tile([P, NB, lc], F8, tag="projT", name=f"projT{b}")
    for eb in range(NB):
        pj = pools["pj_ps"].tile([P, lc], F32, tag="pj", name=f"pj{b}_{eb}")
        for g in range(2):
            nc.tensor.matmul(
                pj[:],
                w8[:, 2 * g : 2 * g + 2, eb * P : (eb + 1) * P],
                t8[:, 2 * g : 2 * g + 2, :],
                start=(g == 0),
                stop=(g == 1),
                perf_mode=DR,
            )
        nc.scalar.copy(projT[:, eb, :], pj[:])
        if ws_q:
            ws_q.pop(0)()
    st.update(projT=projT)


def _stage_mm2(tc, b, st, consts, pools, ws_q):
    """mm2: S[lb] = projT.T @ fT (fp8 DR, contraction e) + row/col max
    reductions; g-groups interleaved across the lb banks so the first
    MMs only need projT eb 0/1."""
    nc = tc.nc
    lc, nbl = st["lc"], st["nbl"]
    f8 = st["f8"]
    projT = st["projT"]

    blk = [min(P, lc - lb * P) for lb in range(nbl)]
    lcp = nbl * P
    s_tiles = [
        pools["s_ps"].tile([blk[lb], lc], F32, tag="s", name=f"s{b}_{lb}")
        for lb in range(nbl)
    ]
    for g in range(2):
        for lb in range(nbl):
            nc.tensor.matmul(
                s_tiles[lb][:],
                projT[:, 2 * g : 2 * g + 2, lb * P : lb * P + blk[lb]],
                f8[:, 2 * g : 2 * g + 2, :],
                start=(g == 0),
                stop=(g == 1),
                perf_mode=DR,
            )
        if ws_q:
            ws_q.pop(0)()
    while ws_q:
        ws_q.pop(0)()

    rm = pools["sv"].tile([P, 2 * nbl], F32, tag="rm", name=f"rm{b}")
    if blk[-1] < P:
        nc.vector.memset(rm[:], 0.0)
    for lb in range(nbl):
        nc.vector.reduce_max(
            rm[0 : blk[lb], lb : lb + 1], s_tiles[lb][:], axis=AX.X
        )
    m1 = pools["m1"].tile([P, lcp], BF16, tag="m1", name=f"m1{b}")
    if lc < lcp:
        nc.vector.memset(m1[:, lc:lcp], -1e30)
    nc.scalar.copy(m1[:, 0:lc], s_tiles[0][:])
    for lb in range(1, nbl):
        nc.vector.tensor_max(
            m1[0 : blk[lb], 0:lc], s_tiles[lb][:], m1[0 : blk[lb], 0:lc]
        )

    st.update(rm=rm, m1=m1, projT=projT)


def _stage_tr(tc, b, st, consts, pools):
    """Colmax transposes + tanh/bias/exp chain (one batch behind)."""
    nc = tc.nc
    nbl = st["nbl"]
    rm, m1 = st["rm"], st["m1"]

    m1t = pools["m1t_ps"].tile([P, nbl, P], BF16, tag="m1t", name=f"m1t{b}")
    for mb in range(nbl):
        nc.tensor.transpose(
            m1t[:, mb, :], m1[:, mb * P : (mb + 1) * P], consts["ident"][:]
        )
    nc.vector.reduce_max(rm[:, nbl : 2 * nbl], m1t[:], axis=AX.X)

    th = pools["sv"].tile([P, 2 * nbl], F32, tag="th", name=f"th{b}")
    nc.scalar.activation(th[:], rm[:], AF.Tanh)
    tb = pools["sv"].tile([P, 2 * nbl], F32, tag="tb", name=f"tb{b}")
    nc.vector.tensor_add(tb[:], th[:], consts["bias_tf"][:, b, :])
    ex = pools["sv"].tile([P, 2 * nbl], BF16, tag="ex", name=f"ex{b}")
    nc.scalar.activation(ex[:], tb[:], AF.Exp)
    st.update(ex=ex)


def _stage_sums(tc, b, st, consts, pools):
    """Sum-of-exp matmul (ones[128,128] stationary -> every partition
    holds the sums), then prescale the exp weights by 1/sum so t+f
    weighted sums can share one accumulating psum region."""
    nc = tc.nc
    nbl = st["nbl"]
    ex = st["ex"]

    sm = pools["sm_ps"].tile([P, 2 * nbl], F32, tag="m1t", name=f"sm{b}")
    nc.tensor.matmul(sm[:], consts["ones_sq"][:], ex[:], start=True, stop=True)
    recb = pools["sv"].tile([P, 2], F32, tag="recb", name=f"recb{b}")
    nc.vector.reduce_sum(
        recb[:], sm[:].rearrange("p (g k) -> p g k", k=nbl), axis=AX.X
    )
    nc.vector.reciprocal(recb[:], recb[:])
    exs = pools["sv"].tile([P, 2 * nbl], BF16, tag="exs", name=f"exs{b}")
    nc.scalar.mul(exs[:, 0:nbl], ex[:, 0:nbl], recb[:, 0:1])
    nc.scalar.mul(exs[:, nbl : 2 * nbl], ex[:, nbl : 2 * nbl], recb[:, 1:2])
    fin = pools["fin_ps"].tile([1, D], F32, tag="fin", name=f"fin{b}")
    st.update(fin=fin, exs=exs)


def _ws_emitters(tc, b, st):
    """The 6 weighted-sum matmuls for batch b as closures, to be
    interleaved between score MMs: score DR MMs are LDWEIGHTS-bound
    (LDW ~184ns > MM ~151ns); a ws MM between two of them gives the
    background weight buffer time to prefetch, making both MM-bound."""
    nc = tc.nc
    nbl = st["nbl"]
    exs, nat, fin = st["exs"], st["nat"], st["fin"]
    n_mm = 2 * nbl

    def mk(tf, lb, k):
        def emit():
            # each ws MM is its own start/stop "group" (accumulation via
            # has_written persists across groups) so the scheduler can
            # interleave them between LDW-bound score MMs
            nc.tensor.matmul(
                fin[0:1, :],
                exs[:, tf * nbl + lb : tf * nbl + lb + 1],
                nat[:, tf, lb, :],
                start=(k == 0),
                stop=True,
                skip_group_check=True,
            )
        return emit

    return [mk(tf, lb, tf * nbl + lb) for tf in range(2) for lb in range(nbl)]


def _stage_out(tc, b, st, consts, pools, o_d):
    nc = tc.nc
    fin = st["fin"]
    nc.scalar.copy(consts["out_acc"][:, b, :], fin[0:1, :])
    nc.sync.dma_start(o_d.ap()[b], consts["out_acc"][0:1, b, :])


_NC_CACHE = {}


def _get_nc(lc):
    if lc not in _NC_CACHE:
        _NC_CACHE[lc] = _build(lc)
    return _NC_CACHE[lc]


def _prep_host(t, f, mask_t, mask_f, w_beta, lc):
    """Compact unmasked rows to the front (padding to lc with zeros),
    then marshal into the device wire formats."""
    t = np.asarray(t, dtype=np.float32)
    f = np.asarray(f, dtype=np.float32)
    w = np.asarray(w_beta, dtype=np.float32)
    e4 = ml_dtypes.float8_e4m3
    nbl = (lc + P - 1) // P
    lcp = nbl * P

    def compact(x, mask):
        xc = np.zeros((B, lcp, D), np.float32)
        biasc = np.full((B, lcp), -BIG, np.float32)
        for b in range(B):
            idx = np.nonzero(mask[b])[0]
            xc[b, : len(idx)] = x[b, idx]
            biasc[b, : len(idx)] = 0.0
        return xc, biasc

    tc_, bias_t = compact(t, np.asarray(mask_t))
    fc_, bias_f = compact(f, np.asarray(mask_f))

    # [p, b, kb, l] = x[b, l, kb*128+p]  (contraction-major for mm1/mm2)
    def to_T8(x):
        x8 = np.clip(x[:, :lc], -240, 240).astype(e4)
        return np.ascontiguousarray(
            x8.transpose(2, 0, 1).reshape(NB, P, B, lc).transpose(1, 2, 0, 3)
        )

    # [p, b, lb, d] = x[b, lb*128+p, d]  (natural for weighted sums)
    def to_nat(x):
        xb = x.astype(ml_dtypes.bfloat16)
        return np.ascontiguousarray(
            xb.transpose(1, 0, 2).reshape(nbl, P, B, D).transpose(1, 2, 0, 3)
        )

    tT8, fT8 = to_T8(tc_), to_T8(fc_)
    tbf, fbf = to_nat(tc_), to_nat(fc_)
    w8 = np.ascontiguousarray(
        np.clip(w, -240, 240).astype(e4).reshape(NB, P, D).transpose(1, 0, 2)
    )
    bias = np.empty((P, B, 2 * nbl), np.float32)
    bias[:, :, 0:nbl] = bias_t.T.reshape(nbl, P, B).transpose(1, 2, 0)
    bias[:, :, nbl:] = bias_f.T.reshape(nbl, P, B).transpose(1, 2, 0)
    return tT8, fT8, tbf, fbf, w8, bias


def _pick_lc(mask_t, mask_f):
    n_max = max(
        int(np.asarray(mask_t).sum(axis=1).max()),
        int(np.asarray(mask_f).sum(axis=1).max()),
    )
    return min(LT, max(P, LC_GRAN * ((n_max + LC_GRAN - 1) // LC_GRAN)))


def _device_inputs(t, f, mask_t, mask_f, w_beta, lc=None):
    if lc is None:
        lc = _pick_lc(mask_t, mask_f)
    tT8, fT8, tbf, fbf, w8, bias = _prep_host(t, f, mask_t, mask_f, w_beta, lc)
    in_maps = []
    for c in range(N_CORES):
        sl = slice(c * BL, (c + 1) * BL)
        in_maps.append(
            {
                "tT8": tT8[:, sl], "fT8": fT8[:, sl],
                "tbf": tbf[:, sl], "fbf": fbf[:, sl],
                "w8": w8, "bias_tf": bias[:, sl],
            }
        )
    return in_maps


def kernel(t, f, mask_t, mask_f, w_beta, **_):
    lc = _pick_lc(mask_t, mask_f)
    nc = _get_nc(lc)
    in_maps = _device_inputs(t, f, mask_t, mask_f, w_beta, lc)
    res = run_bass_kernel_spmd(nc, in_maps, core_ids=list(range(N_CORES)))
    return np.concatenate([r["out"] for r in res.results], axis=0)


if __name__ == "__main__":
    rng = np.random.default_rng(0)
    t = rng.standard_normal((B, LT, D), dtype=np.float32)
    f = rng.standard_normal((B, LF, D), dtype=np.float32)
    mask_t = rng.integers(0, 2, (B, LT)).astype(bool)
    mask_f = rng.integers(0, 2, (B, LF)).astype(bool)
    w_beta = (rng.standard_normal((D, D)) * 0.05).astype(np.float32)
    out = kernel(t=t, f=f, mask_t=mask_t, mask_f=mask_f, w_beta=w_beta)
    print("out", out.shape, out.dtype, np.abs(out).mean())


# revision 35
# speedup vs baseline: 1.0059x; 1.0059x over previous
"""CoAttention kernel for Trainium2 (8 NeuronCores, batch-parallel).

Math (per batch b):
    tm = t * mask_t[:, None]; fm = f * mask_f[:, None]
    S  = (tm @ W) @ fm.T                      # [LT, LF] bilinear scores
    alpha_t = softmax(tanh(rowmax(S)) + (mask_t-1)*BIG)
    alpha_f = softmax(tanh(colmax(S)) + (mask_f-1)*BIG)
    out = alpha_t @ tm + alpha_f @ fm

Bounded-error transformations (verified ~2.4e-3 relative vs the fp32
reference, tolerance 2e-2):
  - tanh is monotonic -> maxes taken on raw S, tanh applied to the
    max vectors only.
  - masking folds into the softmax bias: masked rows get bias -80 ->
    weight ~5e-35, which also covers the final weighted sums.
  - softmax max-subtraction dropped (tanh bounds values in [-1, 1]).
  - the score chain runs fp8-e4m3 with DoubleRow (2 MACs/cell/cycle):
    S only matters through "does tanh saturate", so fp8 noise is
    invisible; measured error identical to a bf16 score chain.
  - weighted sums stay bf16 (that's where all the error comes from).
  - masked rows are compacted out on the host (they are exact zeros in
    the reference's C and carry ~5e-35 softmax weight): unmasked rows
    pack to the front, padded with zero rows + bias -80 to lc =
    64*ceil(max_count/64) (320 for ~50% masks). Zero padding rows
    reproduce exactly the 0 entries the reference's masked rows
    contribute to the row/col maxes; a partial last 64-block is handled
    with partial-partition tiles (rm memset + m1 tail = -1e30 keep the
    unused lanes inert). The kernel variant for each lc is compiled
    lazily, so any mask density (up to full 512) stays correct.

Host-side prep is layout/dtype marshalling + the compaction gather;
all O(B*L*L*D) compute stays on device.

Per-core schedule (8 batches, software-pipelined), per iteration:
  PE: mm1(b) 8 DR MMs -> mm2(b) 6 DR MMs (g-interleaved across banks
  so projT evac latency hides) | colmax transposes(b-1) | weighted
  sums(b-2, bf16) | sum-of-exp matmul(b-1, ones[128,128] stationary so
  every partition holds the sums -> reciprocal is born replicated).
  DVE: per-tile rowmax + colmax max chain straight from PSUM.
  ACT: projT evac fp32->fp8, colmax chain seed, tanh, exp, 1/sum
  prescale, final out copy.
  All DMas are wide linear copies on one queue in priority order.
"""

import numpy as np
import ml_dtypes

import concourse.bass as bass
import concourse.tile as tile
from concourse import bacc, mybir
from concourse import masks as cmasks
from concourse.bass_utils import run_bass_kernel_spmd

F32 = mybir.dt.float32
BF16 = mybir.dt.bfloat16
F8 = mybir.dt.float8e4
AX = mybir.AxisListType
AF = mybir.ActivationFunctionType
DR = mybir.MatmulPerfMode.DoubleRow

N_CORES = 8
B, LT, LF, D = 64, 512, 512, 512
BL = B // N_CORES          # batches per core
P = 128                    # partitions
NB = D // P                # d-dim 128-blocks
LC_GRAN = 64               # compaction granularity (last block may be 64)
BIG = 80.0                 # mask bias (exp(-79) ~ 5e-35; ref uses 1e6, same result)


def _build(lc):
    nbl = (lc + P - 1) // P
    nc = bacc.Bacc("TRN2", target_bir_lowering=False, debug=False, num_devices=N_CORES)

    tT8_d = nc.dram_tensor("tT8", [P, BL, NB, lc], F8, kind="ExternalInput")
    fT8_d = nc.dram_tensor("fT8", [P, BL, NB, lc], F8, kind="ExternalInput")
    tbf_d = nc.dram_tensor("tbf", [P, BL, nbl, D], BF16, kind="ExternalInput")
    fbf_d = nc.dram_tensor("fbf", [P, BL, nbl, D], BF16, kind="ExternalInput")
    w8_d = nc.dram_tensor("w8", [P, NB, D], F8, kind="ExternalInput")
    bias_d = nc.dram_tensor("bias_tf", [P, BL, 2 * nbl], F32, kind="ExternalInput")
    o_d = nc.dram_tensor("out", [BL, D], F32, kind="ExternalOutput")

    with tile.TileContext(nc) as tc:
        _emit(tc, lc, tT8_d, fT8_d, tbf_d, fbf_d, w8_d, bias_d, o_d)
    nc.compile()
    return nc


def _emit(tc, lc, tT8_d, fT8_d, tbf_d, fbf_d, w8_d, bias_d, o_d):
    nc = tc.nc
    nbl = (lc + P - 1) // P
    with (
        tc.tile_pool(name="const", bufs=1) as cpool,
        tc.tile_pool(name="tf8", bufs=BL) as tf8_pool,
        tc.tile_pool(name="nat", bufs=BL) as nat_pool,
        tc.tile_pool(name="pjsb", bufs=2) as pj_sb_pool,
        tc.tile_pool(name="m1", bufs=2) as m1_pool,
        tc.tile_pool(name="sv", bufs=6) as sv_pool,
        tc.tile_pool(name="pjps", bufs=2, space="PSUM") as pj_ps_pool,
        tc.tile_pool(name="sps", bufs=4, space="PSUM") as s_ps_pool,
        tc.tile_pool(name="mtps", bufs=1, space="PSUM") as m1t_ps_pool,
        tc.tile_pool(name="fin", bufs=1, space="PSUM") as fin_ps_pool,
    ):
        pools = dict(
            tf8=tf8_pool, nat=nat_pool, pjsb=pj_sb_pool, m1=m1_pool,
            sv=sv_pool, pj_ps=pj_ps_pool, s_ps=s_ps_pool,
            m1t_ps=m1t_ps_pool, sm_ps=m1t_ps_pool, fin_ps=fin_ps_pool,
        )
        st = [dict(lc=lc, nbl=nbl) for _ in range(BL)]

        ident = cpool.tile([P, P], BF16)
        cmasks.make_identity(nc, ident[:])
        ones_sq = cpool.tile([P, P], BF16)
        nc.vector.memset(ones_sq[:], 1.0)

        # two DMA issue queues in explicit priority order: sync carries
        # the t-side, gpsimd the f-side, so batch 0's pair lands in half
        # the time; nat slabs (needed 2 stages later) trail by 2 batches.
        w8 = cpool.tile([P, NB, D], F8)
        nc.sync.dma_start(w8[:], w8_d.ap())
        bias_tf = cpool.tile([P, BL, 2 * nbl], F32)
        for b in range(BL):
            t8 = tf8_pool.tile([P, NB, lc], F8, tag="t8", name=f"t8_{b}")
            f8 = tf8_pool.tile([P, NB, lc], F8, tag="f8", name=f"f8_{b}")
            st[b]["t8"], st[b]["f8"] = t8, f8
            nat = nat_pool.tile([P, 2, nbl, D], BF16, tag="nat", name=f"nat_{b}")
            st[b]["nat"] = nat
        for b in range(BL):
            nc.sync.dma_start(st[b]["t8"][:], tT8_d.ap()[:, b])
            nc.sync.dma_start(st[b]["f8"][:], fT8_d.ap()[:, b])
            if b == 1:
                nc.sync.dma_start(bias_tf[:], bias_d.ap())
            if b >= 2:
                nat = st[b - 2]["nat"]
                nc.sync.dma_start(nat[:, 0], tbf_d.ap()[:, b - 2])
                nc.sync.dma_start(nat[:, 1], fbf_d.ap()[:, b - 2])
        for b in range(BL - 2, BL):
            nat = st[b]["nat"]
            nc.sync.dma_start(nat[:, 0], tbf_d.ap()[:, b])
            nc.sync.dma_start(nat[:, 1], fbf_d.ap()[:, b])

        consts = dict(w8=w8, ident=ident, ones_sq=ones_sq, bias_tf=bias_tf)
        out_acc = cpool.tile([1, BL, D], F32)
        consts["out_acc"] = out_acc

        for b in range(BL):
            ws_q = _ws_emitters(tc, b - 2, st[b - 2]) if b >= 2 else []
            _stage_mm1(tc, b, st[b], consts, pools, ws_q)
            _stage_mm2(tc, b, st[b], consts, pools, ws_q)
            if b >= 1:
                _stage_tr(tc, b - 1, st[b - 1], consts, pools)
            if b >= 2:
                _stage_out(tc, b - 2, st[b - 2], consts, pools, o_d)
            if b >= 1:
                _stage_sums(tc, b - 1, st[b - 1], consts, pools)
        _stage_tr(tc, BL - 1, st[BL - 1], consts, pools)
        for emit in _ws_emitters(tc, BL - 2, st[BL - 2]):
            emit()
        _stage_out(tc, BL - 2, st[BL - 2], consts, pools, o_d)
        _stage_sums(tc, BL - 1, st[BL - 1], consts, pools)
        for emit in _ws_emitters(tc, BL - 1, st[BL - 1]):
            emit()
        _stage_out(tc, BL - 1, st[BL - 1], consts, pools, o_d)


def _stage_mm1(tc, b, st, consts, pools, ws_q):
    """mm1: projT[e, l] = W.T @ tT (fp8 DoubleRow, contraction d)."""
    nc = tc.nc
    lc, nbl = st["lc"], st["nbl"]
    w8 = consts["w8"]
    t8 = st["t8"]

    projT = pools["pjsb"].tile([P, NB, lc], F8, tag="projT", name=f"projT{b}")
    for eb in range(NB):
        pj = pools["pj_ps"].tile([P, lc], F32, tag="pj", name=f"pj{b}_{eb}")
        for g in range(2):
            nc.tensor.matmul(
                pj[:],
                w8[:, 2 * g : 2 * g + 2, eb * P : (eb + 1) * P],
                t8[:, 2 * g : 2 * g + 2, :],
                start=(g == 0),
                stop=(g == 1),
                perf_mode=DR,
            )
        nc.scalar.copy(projT[:, eb, :], pj[:])
        if ws_q:
            ws_q.pop(0)()
    st.update(projT=projT)


def _stage_mm2(tc, b, st, consts, pools, ws_q):
    """mm2: S[lb] = projT.T @ fT (fp8 DR, contraction e) + row/col max
    reductions; g-groups interleaved across the lb banks so the first
    MMs only need projT eb 0/1."""
    nc = tc.nc
    lc, nbl = st["lc"], st["nbl"]
    f8 = st["f8"]
    projT = st["projT"]

    blk = [min(P, lc - lb * P) for lb in range(nbl)]
    lcp = nbl * P
    s_tiles = [
        pools["s_ps"].tile([blk[lb], lc], F32, tag="s", name=f"s{b}_{lb}")
        for lb in range(nbl)
    ]
    for g in range(2):
        for lb in range(nbl):
            nc.tensor.matmul(
                s_tiles[lb][:],
                projT[:, 2 * g : 2 * g + 2, lb * P : lb * P + blk[lb]],
                f8[:, 2 * g : 2 * g + 2, :],
                start=(g == 0),
                stop=(g == 1),
                perf_mode=DR,
            )
        if ws_q:
            ws_q.pop(0)()
    while ws_q:
        ws_q.pop(0)()

    rm = pools["sv"].tile([P, 2 * nbl], F32, tag="rm", name=f"rm{b}")
    if blk[-1] < P:
        nc.vector.memset(rm[:], 0.0)
    for lb in range(nbl):
        nc.vector.reduce_max(
            rm[0 : blk[lb], lb : lb + 1], s_tiles[lb][:], axis=AX.X
        )
    m1 = pools["m1"].tile([P, lcp], BF16, tag="m1", name=f"m1{b}")
    if lc < lcp:
        nc.vector.memset(m1[:, lc:lcp], -1e30)
    nc.scalar.copy(m1[:, 0:lc], s_tiles[0][:])
    for lb in range(1, nbl):
        nc.vector.tensor_max(
            m1[0 : blk[lb], 0:lc], s_tiles[lb][:], m1[0 : blk[lb], 0:lc]
        )

    st.update(rm=rm, m1=m1, projT=projT)


def _stage_tr(tc, b, st, consts, pools):
    """Colmax transposes + tanh/bias/exp chain (one batch behind)."""
    nc = tc.nc
    nbl = st["nbl"]
    rm, m1 = st["rm"], st["m1"]

    m1t = pools["m1t_ps"].tile([P, nbl, P], BF16, tag="m1t", name=f"m1t{b}")
    for mb in range(nbl):
        nc.tensor.transpose(
            m1t[:, mb, :], m1[:, mb * P : (mb + 1) * P], consts["ident"][:]
        )
    nc.vector.reduce_max(rm[:, nbl : 2 * nbl], m1t[:], axis=AX.X)

    th = pools["sv"].tile([P, 2 * nbl], F32, tag="th", name=f"th{b}")
    nc.scalar.activation(th[:], rm[:], AF.Tanh)
    tb = pools["sv"].tile([P, 2 * nbl], F32, tag="tb", name=f"tb{b}")
    nc.vector.tensor_add(tb[:], th[:], consts["bias_tf"][:, b, :])
    ex = pools["sv"].tile([P, 2 * nbl], BF16, tag="ex", name=f"ex{b}")
    nc.scalar.activation(ex[:], tb[:], AF.Exp)
    st.update(ex=ex)


def _stage_sums(tc, b, st, consts, pools):
    """Sum-of-exp matmul (ones[128,128] stationary -> every partition
    holds the sums), then prescale the exp weights by 1/sum so t+f
    weighted sums can share one accumulating psum region."""
    nc = tc.nc
    nbl = st["nbl"]
    ex = st["ex"]

    sm = pools["sm_ps"].tile([P, 2 * nbl], F32, tag="m1t", name=f"sm{b}")
    nc.tensor.matmul(sm[:], consts["ones_sq"][:], ex[:], start=True, stop=True)
    recb = pools["sv"].tile([P, 2], F32, tag="recb", name=f"recb{b}")
    nc.vector.reduce_sum(
        recb[:], sm[:].rearrange("p (g k) -> p g k", k=nbl), axis=AX.X
    )
    nc.vector.reciprocal(recb[:], recb[:])
    exs = pools["sv"].tile([P, 2 * nbl], BF16, tag="exs", name=f"exs{b}")
    nc.scalar.mul(exs[:, 0:nbl], ex[:, 0:nbl], recb[:, 0:1])
    nc.scalar.mul(exs[:, nbl : 2 * nbl], ex[:, nbl : 2 * nbl], recb[:, 1:2])
    fin = pools["fin_ps"].tile([1, D], F32, tag="fin", name=f"fin{b}")
    st.update(fin=fin, exs=exs)


def _ws_emitters(tc, b, st):
    """The 6 weighted-sum matmuls for batch b as closures, to be
    interleaved between score MMs: score DR MMs are LDWEIGHTS-bound
    (LDW ~184ns > MM ~151ns); a ws MM between two of them gives the
    background weight buffer time to prefetch, making both MM-bound."""
    nc = tc.nc
    nbl = st["nbl"]
    exs, nat, fin = st["exs"], st["nat"], st["fin"]
    n_mm = 2 * nbl

    def mk(tf, lb, k):
        def emit():
            nc.tensor.matmul(
                fin[0:1, :],
                exs[:, tf * nbl + lb : tf * nbl + lb + 1],
                nat[:, tf, lb, :],
                start=(k == 0),
                stop=(k == n_mm - 1),
            )
        return emit

    return [mk(tf, lb, tf * nbl + lb) for tf in range(2) for lb in range(nbl)]


def _stage_out(tc, b, st, consts, pools, o_d):
    nc = tc.nc
    fin = st["fin"]
    nc.scalar.copy(consts["out_acc"][:, b, :], fin[0:1, :])
    nc.sync.dma_start(o_d.ap()[b], consts["out_acc"][0:1, b, :])


_NC_CACHE = {}


def _get_nc(lc):
    if lc not in _NC_CACHE:
        _NC_CACHE[lc] = _build(lc)
    return _NC_CACHE[lc]


def _prep_host(t, f, mask_t, mask_f, w_beta, lc):
    """Compact unmasked rows to the front (padding to lc with zeros),
    then marshal into the device wire formats."""
    t = np.asarray(t, dtype=np.float32)
    f = np.asarray(f, dtype=np.float32)
    w = np.asarray(w_beta, dtype=np.float32)
    e4 = ml_dtypes.float8_e4m3
    nbl = (lc + P - 1) // P
    lcp = nbl * P

    def compact(x, mask):
        xc = np.zeros((B, lcp, D), np.float32)
        biasc = np.full((B, lcp), -BIG, np.float32)
        for b in range(B):
            idx = np.nonzero(mask[b])[0]
            xc[b, : len(idx)] = x[b, idx]
            biasc[b, : len(idx)] = 0.0
        return xc, biasc

    tc_, bias_t = compact(t, np.asarray(mask_t))
    fc_, bias_f = compact(f, np.asarray(mask_f))

    # [p, b, kb, l] = x[b, l, kb*128+p]  (contraction-major for mm1/mm2)
    def to_T8(x):
        x8 = np.clip(x[:, :lc], -240, 240).astype(e4)
        return np.ascontiguousarray(
            x8.transpose(2, 0, 1).reshape(NB, P, B, lc).transpose(1, 2, 0, 3)
        )

    # [p, b, lb, d] = x[b, lb*128+p, d]  (natural for weighted sums)
    def to_nat(x):
        xb = x.astype(ml_dtypes.bfloat16)
        return np.ascontiguousarray(
            xb.transpose(1, 0, 2).reshape(nbl, P, B, D).transpose(1, 2, 0, 3)
        )

    tT8, fT8 = to_T8(tc_), to_T8(fc_)
    tbf, fbf = to_nat(tc_), to_nat(fc_)
    w8 = np.ascontiguousarray(
        np.clip(w, -240, 240).astype(e4).reshape(NB, P, D).transpose(1, 0, 2)
    )
    bias = np.empty((P, B, 2 * nbl), np.float32)
    bias[:, :, 0:nbl] = bias_t.T.reshape(nbl, P, B).transpose(1, 2, 0)
    bias[:, :, nbl:] = bias_f.T.reshape(nbl, P, B).transpose(1, 2, 0)
    return tT8, fT8, tbf, fbf, w8, bias


def _pick_lc(mask_t, mask_f):
    n_max = max(
        int(np.asarray(mask_t).sum(axis=1).max()),
        int(np.asarray(mask_f).sum(axis=1).max()),
    )
    return min(LT, max(P, LC_GRAN * ((n_max + LC_GRAN - 1) // LC_GRAN)))


def _device_inputs(t, f, mask_t, mask_f, w_beta, lc=None):
    if lc is None:
        lc = _pick_lc(mask_t, mask_f)
    tT8, fT8, tbf, fbf, w8, bias = _prep_host(t, f, mask_t, mask_f, w_beta, lc)
    in_maps = []
    for c in range(N_CORES):
        sl = slice(c * BL, (c + 1) * BL)
        in_maps.append(
            {
                "tT8": tT8[:, sl], "fT8": fT8[:, sl],
                "tbf": tbf[:, sl], "fbf": fbf[:, sl],
                "w8": w8, "bias_tf": bias[:, sl],
            }
        )
    return in_maps


def kernel(t, f, mask_t, mask_f, w_beta, **_):
    lc = _pick_lc(mask_t, mask_f)
    nc = _get_nc(lc)
    in_maps = _device_inputs(t, f, mask_t, mask_f, w_beta, lc)
    res = run_bass_kernel_spmd(nc, in_maps, core_ids=list(range(N_CORES)))
    return np.concatenate([r["out"] for r in res.results], axis=0)


if __name__ == "__main__":
    rng = np.random.default_rng(0)
    t = rng.standard_normal((B, LT, D), dtype=np.float32)
    f = rng.standard_normal((B, LF, D), dtype=np.float32)
    mask_t = rng.integers(0, 2, (B, LT)).astype(bool)
    mask_f = rng.integers(0, 2, (B, LF)).astype(bool)
    w_beta = (rng.standard_normal((D, D)) * 0.05).astype(np.float32)
    out = kernel(t=t, f=f, mask_t=mask_t, mask_f=mask_f, w_beta=w_beta)
    print("out", out.shape, out.dtype, np.abs(out).mean())
